# revision 90
# baseline (speedup 1.0000x reference)
"""Trainium2 Bass kernel for MultiHeadCrossAttention.

Problem shapes (hardcoded; see module constants):
  query      [8, 512, 768] f32
  key_value  [8, 2048, 768] f32
  kv_mask    [8, 2048] bool
  Wq/Wk/Wv   [768, 1024] f32, Wo [1024, 1024] f32, biases [1024] f32

Sharding: pure data-parallel -- batch element b runs on core b (8 cores, no
collectives). Each core computes the full attention stack for its batch
element and writes out^T [1024, 512]; the host transposes and stacks.

Host-side prep (layout/dtype prep only):
  - masked-KV compaction: rows with kv_mask=False contribute exactly nothing
    to the output (their softmax weight is exactly 0), so the host gathers
    only the unmasked rows per batch and pads to a multiple of 128
    (LKV_P = nkvc*128, nkvc = ceil(max_count/128), typically 9 -> 1152 of
    2048). Padded rows get mask_bias -30000 => exp == 0 exactly.
  - activations and Wq/Wo converted to bf16; Wk/Wv pre-scaled by 64 and
    split into fp8e4 hi/lo pairs (DoubleRow packing); kv_mask folded to an
    additive bias vector; bv folded into bo (exact since softmax rows sum
    to 1: out += bv @ Wo); all bias vectors packed into one tensor.
  - weight tensors are blocked/packed so each DMA lands in its exact SBUF
    layout with few large transfers (HWDGE has ~630ns fixed cost per DMA).

Per-core dataflow (fp32 PSUM accumulation everywhere):
  - q^T [768,512], kv^T [768,LKV_P] built by DMA loads + TensorE transpose
    (identity matmul) + DVE copy, batched up to 4 row-tiles per psum tile.
  - kv^T additionally split into fp8e4 hi/lo DoubleRow-packed tiles
    (hi: ScalarE copies, lo: subtracts alternating DVE/Pool).
  - Q^T = Wq^T @ q^T [1024,512] in bf16 (tiles 0-1 up front, the rest as
    attention fill work).
  - K^T = Wk^T @ kv^T [1024,LKV_P] and V = kv @ Wv [LKV_P,1024] as fp8
    DoubleRow matmuls (2 contraction rows/pass at 0.5 cycles/row) with
    3 hi/lo cross terms -- ~25% cheaper than bf16 and slightly more
    accurate; the /64 weight pre-scale is undone in the PSUM evacuation.
    V is stored head-interleaved with an appended ones-column
    ([128, 16, 65] tiles): each head's O-matmul then also produces the
    softmax denominator for free.
  - Attention per head pair (2t, 2t+1): S^T pair psum [128, 1024] per kv
    chunk of 128 (chunk x 2 heads), exp on ScalarE with scale=1/8 and the
    kv-mask as per-partition bias (-30000 => exp==0); no max-subtraction
    (scores are O(1) by construction, exp cannot overflow).
  - O accumulated in [q, d] layout (q on partitions): stationary = P^T
    chunk [128kv, 128q], moving = V head slice [128kv, 65] -> psum
    [128q, 65] per (sub, qtile); col 64 is the denominator. This makes the
    normalization a per-partition scalar op (DVE reciprocal + multiply)
    and the O matmuls cost 65-wide moving streams instead of 512. A psum
    start marks the whole 2KB zero-region, so the four [128,65] groups
    in one bank share a single start/stop.
  - O_norm [128q, 64] tiles are PE-transposed back to O^T [128, 512] per
    pair (8 transposes + one DVE copy) for the output projection.
  - out^T = Wo^T @ O^T + bo_eff in accumulation phases (kt 0-3 + bias in
    pairs 4-5, kt 4-5 in pair 6, kt 6 in pair 7, kt 7 in the tail)
    combined by DVE adds; bf16 output stores batched two blocks per DMA
    on alternating queues.
  - K^T/Q^T projections for later pairs, V chunks + deferred fp8
    conversions (pair 0), and the out-projection phases are emitted
    between each slot's S matmuls and O matmuls so the TensorE always has
    fill work while ScalarE runs the exps.
"""

import numpy as np
import ml_dtypes

import concourse.bass as bass
import concourse.bacc as bacc
import concourse.mybir as mybir
import concourse.tile as tile
from concourse.bass_utils import run_bass_kernel_spmd

dt = mybir.dt
AF = mybir.ActivationFunctionType

B = 8
LQ = 512
QD = 768
HID = 1024
H = 16
DH = 64
SCALE = DH**-0.5
MASK_NEG = -30000.0

F32 = dt.float32
BF16 = dt.bfloat16

NQT = QD // 128  # 6 feature tiles
NLQ = LQ // 128  # 4 query-row tiles
NH = HID // 128  # 8 hidden tiles (= head pairs)

DEFAULT_NKVC = 9  # ceil(~1030/128)+margin; recompiled on demand if exceeded


def build_nc(nkvc):
    lkv = nkvc * 128
    nc = bacc.Bacc("TRN2", target_bir_lowering=False, debug=False)

    q_d = nc.dram_tensor("q_bf", [LQ, QD], BF16, kind="ExternalInput")
    kv_d = nc.dram_tensor("kv_bf", [lkv, QD], BF16, kind="ExternalInput")
    # Wq/Wk host-packed into per-output-block tensors ([NH][128, 768]:
    # block mt row p col (kt*128+m) = W[kt*128+p, mt*128+m]) so the block
    # that gates the first attention pair is a single small early DMA;
    # Wv host-packed into two per-head-half tensors ([2][128, 6*512]).
    # Wk / Wv in fp8e4 hi/lo pairs, DoubleRow-packed: the K and V
    # projections run as fp8 DoubleRow matmuls (2 contraction rows per
    # pass, 0.5 cycles/row) with an exact-ish hi+lo split of both
    # operands (3 cross terms; dropped lo*lo is ~2^-16), which is both
    # faster than bf16 and slightly more accurate.
    # large-nkvc fallback (essentially never taken: needs a kv_mask with
    # >1536 set bits in some batch) drops the fp8 path to fit SBUF
    use_fp8 = nkvc <= 12
    wqb_d = nc.dram_tensor("Wq_blk", [NH, 128, QD], BF16, kind="ExternalInput")
    if use_fp8:
        wkb_d = nc.dram_tensor(
            "Wk_blk8", [NH, 128, 2, NQT // 2, 2, 128], dt.float8e4,
            kind="ExternalInput",
        )
        wvh_d = nc.dram_tensor(
            "Wv_half8", [2, 128, 2, NQT // 2, 2, 512], dt.float8e4,
            kind="ExternalInput",
        )
    else:
        wkb_d = nc.dram_tensor(
            "Wk_blk", [NH, 128, QD], BF16, kind="ExternalInput"
        )
        wvh_d = nc.dram_tensor(
            "Wv_half", [2, 128, NQT * 512], BF16, kind="ExternalInput"
        )
    wo_d = nc.dram_tensor("Wo_bf", [HID, HID], BF16, kind="ExternalInput")
    # all bias vectors host-packed into one [3*NH+nkvc, 128] tensor
    # (rows: bq, mask_bias, bk, bo_eff) so one DMA + one PE transpose
    # produces every per-partition bias column
    nbias = 3 * NH + nkvc
    bias_d = nc.dram_tensor("biasvec", [nbias, 128], F32, kind="ExternalInput")
    id_d = nc.dram_tensor("ident", [128, 128], BF16, kind="ExternalInput")
    # bf16 output halves the serial output-writeback DMA in the tail; the
    # host upcasts. Costs ~0.2% extra rel err on a 2% budget.
    out_d = nc.dram_tensor("out", [HID, LQ], BF16, kind="ExternalOutput")

    with tile.TileContext(nc) as tc:
        with (
            tc.tile_pool(name="persist", bufs=1) as persist,
            tc.tile_pool(name="stage", bufs=6) as stage,
            tc.tile_pool(name="ppool", bufs=4 if nkvc <= 12 else 3) as ppool,
            tc.tile_pool(name="finpool", bufs=2) as finpool,
            tc.tile_pool(name="spsum", bufs=2, space="PSUM") as spsum,
            tc.tile_pool(name="opsum", bufs=1, space="PSUM") as opsum,
            tc.tile_pool(name="fpsum", bufs=2, space="PSUM") as fpsum,
        ):
            # ---- loads + PE-based transposes ------------------------------
            qT = [
                persist.tile([128, LQ], BF16, tag=f"qT{ft}", name=f"qT{ft}")
                for ft in range(NQT)
            ]
            kvT = [
                persist.tile([128, lkv], BF16, tag=f"kvT{ft}", name=f"kvT{ft}")
                for ft in range(NQT)
            ]
            ident = persist.tile([128, 128], BF16, tag="ident")
            nc.scalar.dma_start(ident[:], id_d[:])
            # All input DMAs are issued up front, batched into few large
            # transfers (HWDGE is a shared serial resource with ~630ns fixed
            # cost per DMA), ordered per queue by first PE use:
            #   sync:   kv rows (2 DMAs), Wk (2)
            #   scalar: ident, q rows (1), Wv (2)
            #   pool:   Wq (2), biases (1), Wo (2, emitted later)
            def row_block(src, r0, r1):
                return src[r0 * 128 : r1 * 128, :].rearrange(
                    "(t p) m -> p t m", p=128
                )

            # kv staged in one pass; the (essentially never taken) large
            # fallback reuses a 12-tile stage buffer in two passes
            nkv_st = min(nkvc, 12)
            kv_st = stage.tile([128, nkv_st, QD], BF16, tag="stgkv", bufs=1)
            nc.sync.dma_start(kv_st[:, 0:2, :], row_block(kv_d, 0, 2))
            nc.sync.dma_start(
                kv_st[:, 2:nkv_st, :], row_block(kv_d, 2, nkv_st)
            )
            # Wk blocks 0-1 land before Wv: the pair-0 K projection gates
            # the whole attention chain while V only gates O(0)
            wkb = []
            for t in range(NH):
                wkt = persist.tile(
                    [128, 2, NQT // 2, 2, 128] if use_fp8 else [128, QD],
                    dt.float8e4 if use_fp8 else BF16,
                    tag=f"wkb{t}", name=f"wkb{t}",
                )
                if t < 2:
                    nc.sync.dma_start(wkt[:], wkb_d[t])
                wkb.append(wkt)
            wvh = []
            for nh in range(2):
                wvt = persist.tile(
                    [128, 2, NQT // 2, 2, 512] if use_fp8 else [128, NQT * 512],
                    dt.float8e4 if use_fp8 else BF16,
                    tag=f"wvh{nh}", name=f"wvh{nh}",
                )
                nc.sync.dma_start(wvt[:], wvh_d[nh])
                wvh.append(wvt)
            for t in range(2, NH):
                nc.sync.dma_start(wkb[t][:], wkb_d[t])
            q_st = stage.tile([128, NLQ, QD], BF16, tag="stgq", bufs=1)
            nc.scalar.dma_start(q_st[:], row_block(q_d, 0, NLQ))
            wqb = []
            for mt in range(2):
                wqt = persist.tile([128, QD], BF16, tag=f"wqb{mt}", name=f"wqb{mt}")
                nc.gpsimd.dma_start(wqt[:], wqb_d[mt])
                wqb.append(wqt)
            b_st = stage.tile([nbias, 128], F32, tag="bst", name="b_st", bufs=1)
            nc.gpsimd.dma_start(b_st[:], bias_d[:])
            for mt in range(2, NH):
                wqt = persist.tile([128, QD], BF16, tag=f"wqb{mt}", name=f"wqb{mt}")
                nc.gpsimd.dma_start(wqt[:], wqb_d[mt])
                wqb.append(wqt)

            # fp8 hi/lo versions of kv^T, DoubleRow-packed by feature-tile
            # pair, feeding the K and V projections
            kvT8h = [
                persist.tile(
                    [128, 2, lkv], dt.float8e4, tag=f"kv8h{j}", name=f"kv8h{j}"
                )
                for j in range(NQT // 2)
            ] if use_fp8 else []
            kvT8l = [
                persist.tile(
                    [128, 2, lkv], dt.float8e4, tag=f"kv8l{j}", name=f"kv8l{j}"
                )
                for j in range(NQT // 2)
            ] if use_fp8 else []

            def emit_transpose_group(
                dst_tiles, st_tile, lt0, nlt, to_fp8=False, st_off=0
            ):
                for ft in range(NQT):
                    # alternate between the fill pool and the (startup-idle)
                    # S pool so four transpose groups can be in flight
                    pool = fpsum if ft % 2 == 0 else spsum
                    tp = pool.tile(
                        [128, 1024], BF16,
                        tag="fp" if ft % 2 == 0 else "sps", name="tp",
                    )
                    for j in range(nlt):
                        nc.tensor.transpose(
                            tp[:, j * 128 : (j + 1) * 128],
                            st_tile[:, lt0 - st_off + j, ft * 128 : (ft + 1) * 128],
                            ident[:],
                        )
                    nc.vector.tensor_copy(
                        dst_tiles[ft][:, lt0 * 128 : (lt0 + nlt) * 128],
                        tp[:, 0 : nlt * 128],
                    )
                if not to_fp8:
                    return
                # conversions are deferred (emit_kv_fp8) so they don't block
                # the DVE transpose-evacuation pipeline

            # bias block: one PE transpose of the packed [nbias, 128] rows
            # into per-partition [128, nbias] columns
            idf = persist.tile([nbias, nbias], F32, tag="idf")
            nc.vector.tensor_copy(idf[:], ident[0:nbias, 0:nbias])
            idf128 = persist.tile([128, 128], dt.float32r, tag="idf128")
            with nc.allow_low_precision(reason="fp32r identity"):
                nc.vector.tensor_copy(idf128[:], ident[:])

            # kv transpose groups: a small first group so the PE starts as
            # soon as the first kv rows land
            kv_groups = []
            lt0 = 0
            while lt0 < nkvc:
                g = 2 if lt0 == 0 else min(4, nkvc - lt0)
                if lt0 < nkv_st:
                    g = min(g, nkv_st - lt0)  # don't cross the staging split
                kv_groups.append((lt0, min(g, nkvc - lt0)))
                lt0 += kv_groups[-1][1]

            for g in kv_groups:
                if g[0] + g[1] <= nkv_st:
                    emit_transpose_group(kvT, kv_st, *g, to_fp8=True)
            if nkvc > nkv_st:
                # second staging pass over the first slots of the buffer
                nc.sync.dma_start(
                    kv_st[:, 0 : nkvc - nkv_st, :],
                    row_block(kv_d, nkv_st, nkvc),
                )
                for g in kv_groups:
                    if g[0] + g[1] > nkv_st:
                        emit_transpose_group(
                            kvT, kv_st, *g, to_fp8=True, st_off=nkv_st
                        )
            emit_transpose_group(qT, q_st, 0, NLQ)
            # kv^T fp8 hi/lo conversion, in column order so the first K/V
            # projection chunks unblock early: hi on ScalarE, lo alternating
            # DVE/Pool (Pool may not touch PSUM but these are SBUF-only)
            def emit_kv_fp8(groups, lo_pool_ok=False):
                # lo subtracts gate the K projections; only the deferred
                # (pair-0) group may use the slow Pool engine
                if not use_fp8:
                    return
                with nc.allow_low_precision(reason="fp8 hi/lo split"):
                    for lt0, nlt in groups:
                        c0, c1 = lt0 * 128, (lt0 + nlt) * 128
                        for j in range(NQT // 2):
                            for tw in range(2):
                                src = kvT[2 * j + tw][:, c0:c1]
                                nc.scalar.copy(kvT8h[j][:, tw, c0:c1], src)
                                lo_eng = (
                                    nc.gpsimd
                                    if lo_pool_ok and (j + tw) % 2 == 0
                                    else nc.vector
                                )
                                lo_eng.tensor_tensor(
                                    kvT8l[j][:, tw, c0:c1],
                                    src,
                                    kvT8h[j][:, tw, c0:c1],
                                    mybir.AluOpType.subtract,
                                )

            emit_kv_fp8(kv_groups[:2])
            b_all = persist.tile([128, nbias], F32, tag="ball")
            b_ps = fpsum.tile([128, nbias], F32, tag="fp", name="b_ps")
            nc.tensor.transpose(b_ps[:], b_st[:], idf[:])
            nc.vector.tensor_copy(b_all[:], b_ps[:])
            bq_sb = b_all[:, 0:NH]
            mb_sb = b_all[:, NH : NH + nkvc]
            bk_sb = b_all[:, NH + nkvc : 2 * NH + nkvc]
            boe_sb = b_all[:, 2 * NH + nkvc : 3 * NH + nkvc]

            # ---- Q^T projection: [1024, 512] bf16 -------------------------
            # tiles 0-1 up front; 2-7 stream as fill work in pairs 0-3
            # (pair t only needs QT[t]).
            QT = [None] * NH

            # hi/lo cross terms: (w_hi, x_hi), (w_hi, x_lo), (w_lo, x_hi)
            HILO = [(0, 0), (0, 1), (1, 0)]

            def emit_qproj(mt):
                ps = fpsum.tile([128, 512], F32, tag="fp", name="ps")
                qt_t = persist.tile([128, LQ], BF16, tag=f"QT{mt}", name=f"QT{mt}")
                for kt in range(NQT):
                    nc.tensor.matmul(
                        ps[:],
                        wqb[mt][:, kt * 128 : (kt + 1) * 128],
                        qT[kt][:],
                        start=(kt == 0),
                        stop=(kt == NQT - 1),
                    )
                nc.vector.tensor_scalar_add(
                    qt_t[:], ps[:], bq_sb[:, mt : mt + 1]
                )
                QT[mt] = qt_t

            emit_qproj(0)
            emit_qproj(1)
            if not use_fp8:
                # fallback: all Q up front so qT slots can be recycled for OT
                for mt in range(2, NH):
                    emit_qproj(mt)

            # ---- K^T projection (chunks of <=512 kv columns) --------------
            KT = [
                persist.tile([128, lkv], BF16, tag=f"KT{t}", name=f"KT{t}")
                for t in range(NH)
            ]
            # 256-wide chunks make slot-sized (~640ns) fill units that
            # spread across the attention slots of each pair
            kchunks = []
            c0 = 0
            while c0 < lkv:
                kchunks.append((c0, min(256, lkv - c0)))
                c0 += kchunks[-1][1]
            kslots = [1, 3, 5, 7, 8, 2, 4, 6][: len(kchunks)]

            def emit_ktproj(t, c0, csz):
                ps = fpsum.tile([128, 512], F32, tag="fp", name="ps")
                ps = ps[:, 0:csz]
                if use_fp8:
                    kvT8 = [kvT8h, kvT8l]
                    n = 0
                    for j in range(NQT // 2):
                        for wh, xh in HILO:
                            n += 1
                            nc.tensor.matmul(
                                ps[:],
                                wkb[t][:, wh, j, :, :],
                                kvT8[xh][j][:, :, c0 : c0 + csz],
                                start=(n == 1),
                                stop=(n == 3 * (NQT // 2)),
                                perf_mode=mybir.MatmulPerfMode.DoubleRow,
                            )
                    nc.vector.tensor_scalar(
                        KT[t][:, c0 : c0 + csz],
                        ps[:],
                        1.0 / 64.0,
                        bk_sb[:, t : t + 1],
                        mybir.AluOpType.mult,
                        mybir.AluOpType.add,
                    )
                else:
                    for kt in range(NQT):
                        nc.tensor.matmul(
                            ps[:],
                            wkb[t][:, kt * 128 : (kt + 1) * 128],
                            kvT[kt][:, c0 : c0 + csz],
                            start=(kt == 0),
                            stop=(kt == NQT - 1),
                        )
                    nc.vector.tensor_scalar_add(
                        KT[t][:, c0 : c0 + csz], ps[:], bk_sb[:, t : t + 1]
                    )

            # ---- V projection, interleaved [128, 16, 65] with ones col ----
            V_il = [None] * nkvc

            def emit_vproj(lt):
                vt = persist.tile(
                    [128, H, DH + 1], BF16, tag=f"V{lt}", name=f"V{lt}"
                )
                nc.vector.memset(vt[:, :, DH], 1.0)
                kvT8 = [kvT8h, kvT8l]
                for nh in range(2):
                    ps = fpsum.tile([128, 512], F32, tag="fp", name="ps")
                    if use_fp8:
                        n = 0
                        for j in range(NQT // 2):
                            for xh, wh in HILO:
                                n += 1
                                nc.tensor.matmul(
                                    ps[:],
                                    kvT8[xh][j][:, :, lt * 128 : (lt + 1) * 128],
                                    wvh[nh][:, wh, j, :, :],
                                    start=(n == 1),
                                    stop=(n == 3 * (NQT // 2)),
                                    perf_mode=mybir.MatmulPerfMode.DoubleRow,
                                )
                        nc.vector.tensor_scalar_mul(
                            vt[:, nh * 8 : (nh + 1) * 8, 0:DH],
                            ps.rearrange("p (h d) -> p h d", d=DH),
                            1.0 / 64.0,
                        )
                    else:
                        for kt in range(NQT):
                            nc.tensor.matmul(
                                ps[:],
                                kvT[kt][:, lt * 128 : (lt + 1) * 128],
                                wvh[nh][:, kt * 512 : (kt + 1) * 512],
                                start=(kt == 0),
                                stop=(kt == NQT - 1),
                            )
                        nc.vector.tensor_copy(
                            vt[:, nh * 8 : (nh + 1) * 8, 0:DH],
                            ps.rearrange("p (h d) -> p h d", d=DH),
                        )
                V_il[lt] = vt

            # K^T pair 0 up front for the columns whose fp8 conversions ran
            # (kv groups 0-1, cols < 768); the last kv group's conversions
            # and remaining pair-0 chunks stream as early pair-0 fill work.
            k0_defer = [ch for ch in kchunks if ch[0] + ch[1] > 768]
            for ch in kchunks:
                if ch not in k0_defer:
                    emit_ktproj(0, *ch)

            # Wo loads (needed only at the end)
            wo_t = persist.tile([128, NH, HID], BF16, tag="wo")
            nc.gpsimd.dma_start(wo_t[:, 0:4, :], row_block(wo_d, 0, 4))
            nc.gpsimd.dma_start(wo_t[:, 4:8, :], row_block(wo_d, 4, 8))
            wo_bf = [wo_t[:, kt, :] for kt in range(NH)]

            # ---- attention per head pair ---------------------------------
            # output projection in three accumulation phases so only Wo's
            # last slice remains after the final pair:
            #   A: heads 0-7 (kt 0-3) + bias, during pairs 4-5
            #   B: heads 8-13 (kt 4-6) added, during pair 7
            #   C: heads 14-15 (kt 7) added, tail
            outpart = [None] * NH

            def emit_outA(mt):
                ps = fpsum.tile([128, 512], F32, tag="fp", name="ps")
                for kt in range(4):
                    nc.tensor.matmul(
                        ps[:],
                        wo_bf[kt][:, mt * 128 : (mt + 1) * 128],
                        OT[kt][:],
                        start=(kt == 0),
                        stop=(kt == 3),
                    )
                # bf16 partials in the large-nkvc fallback to fit SBUF
                # fp32r so the tail's identity-matmul combine is legal on
                # hw (fp32r consumers need fp32r-rounded producers);
                # bf16 partials in the large-nkvc fallback to fit SBUF
                op_t = persist.tile(
                    [128, 512], dt.float32r if use_fp8 else BF16,
                    tag=f"outpart{mt}", name=f"outpart{mt}",
                )
                with nc.allow_low_precision(reason="fp32r partials"):
                    nc.vector.tensor_scalar_add(
                        op_t[:], ps[:], boe_sb[:, mt : mt + 1]
                    )
                outpart[mt] = op_t

            def emit_outB(mt, k0, k1):
                ps = fpsum.tile([128, 512], F32, tag="fp", name="ps")
                for kt in range(k0, k1):
                    nc.tensor.matmul(
                        ps[:],
                        wo_bf[kt][:, mt * 128 : (mt + 1) * 128],
                        OT[kt][:],
                        start=(kt == k0),
                        stop=(kt == k1 - 1),
                    )
                with nc.allow_low_precision(reason="fp32r partials"):
                    nc.vector.tensor_tensor(
                        outpart[mt][:], ps[:], outpart[mt][:], mybir.AluOpType.add
                    )

            # deferred normalize+transpose state from the previous pair
            pend = {}

            def emit_norm(t, o_ps):
                # per-partition normalize: rc = 1/denom (col 64), then
                # scalar-multiply into bf16 [q, d] tiles. Mid-kernel the
                # multiply runs on DVE (ScalarE is the exp chain); for the
                # last pair ScalarE is free, so it takes the multiplies and
                # pipelines against DVE's reciprocals.
                on_sb = []
                for sub in range(2):
                    rc = finpool.tile([128, 4], F32, tag="rc", name="rc")
                    on = finpool.tile(
                        [128, NLQ, DH], BF16, tag=f"on{sub}", name="on"
                    )
                    for qt in range(NLQ):
                        nc.vector.reciprocal(
                            rc[:, qt : qt + 1], o_ps[sub][:, qt, DH : DH + 1]
                        )
                        nc.vector.tensor_scalar_mul(
                            on[:, qt, :],
                            o_ps[sub][:, qt, 0:DH],
                            rc[:, qt : qt + 1],
                        )
                    on_sb.append(on)
                pend[t] = on_sb

            def emit_otrans(t, half):
                # transpose O_norm [q, d] -> O^T [d, q] on the PE; half 0/1
                # each moves one sub-head (4 transposes); half 1 also copies
                # the completed [128, 512] psum tile into OT[t].
                on_sb = pend[t]
                if half == 0:
                    pend["ot_ps"] = fpsum.tile([128, 512], BF16, tag="fp", name="otp")
                ot_ps = pend["ot_ps"]
                for qt in range(NLQ):
                    nc.tensor.transpose(
                        ot_ps[half * 64 : half * 64 + 64, qt * 128 : (qt + 1) * 128],
                        on_sb[half][:, qt, :],
                        ident[:],
                    )
                if half == 1:
                    ot_t = persist.tile(
                        [128, LQ], BF16,
                        tag=(f"qT{t}" if not use_fp8 and t < NQT else f"OT{t}"),
                        name=f"OT{t}",
                    )
                    nc.vector.tensor_copy(ot_t[:], ot_ps[:])
                    OT.append(ot_t)
                    del pend[t], pend["ot_ps"]

            OT = []
            for t in range(NH):
                o_ps = [
                    opsum.tile([128, NLQ, DH + 1], F32, tag=f"o{sub}", name=f"o{sub}")
                    for sub in range(2)
                ]
                # fill schedule: (slot, thunk) pairs emitted between S(slot)
                # and O(slot), so the PE has fill work while ScalarE exps
                fills = []
                if t == 0:
                    fills.append((1, lambda: emit_kv_fp8(kv_groups[2:], lo_pool_ok=True)))
                    for j, ch in enumerate(k0_defer):
                        # chunk (c0, csz) gates S(0, kc >= c0//128): slot
                        # must come earlier than that
                        fills.append(
                            (
                                min(j + 2, max(0, ch[0] // 128 - 1)),
                                lambda ch=ch: emit_ktproj(0, *ch),
                            )
                        )
                    for lt in range(nkvc):
                        fills.append((lt, lambda lt=lt: emit_vproj(lt)))
                    for j, ch in enumerate(kchunks):
                        fills.append(
                            (min(j + 2, nkvc - 1), lambda ch=ch: emit_ktproj(1, *ch))
                        )
                    if use_fp8:
                        fills.append((7, lambda: emit_qproj(2)))
                else:
                    if use_fp8 and t in (1, 2):
                        fills.append((2, lambda t=t: emit_qproj(2 * t + 1)))
                        fills.append((5, lambda t=t: emit_qproj(2 * t + 2)))
                    if use_fp8 and t == 3:
                        fills.append((2, lambda: emit_qproj(7)))
                    if t >= 1:
                        fills.append((0, lambda t=t: emit_otrans(t - 1, 0)))
                        fills.append((1, lambda t=t: emit_otrans(t - 1, 1)))
                    if t < NH - 1:
                        for j, ch in enumerate(kchunks):
                            fills.append(
                                (
                                    min(kslots[j], nkvc - 1),
                                    lambda ch=ch: emit_ktproj(t + 1, *ch),
                                )
                            )
                    if t in (4, 5):
                        for i in range(4):
                            mt = 4 * (t - 4) + i
                            fills.append((2 * i + 2, lambda mt=mt: emit_outA(mt)))
                    if t == 6:
                        for mt in range(NH):
                            fills.append(
                                (mt // 2 + 2, lambda mt=mt: emit_outB(mt, 4, 6))
                            )
                    if t == NH - 1:
                        for mt in range(NH):
                            fills.append(
                                (mt + 1, lambda mt=mt: emit_outB(mt, 6, 7))
                            )
                fills.sort(key=lambda x: x[0])
                fi = 0

                for kc in range(nkvc):
                    s = spsum.tile([128, 1024], F32, tag="sps", name="s")
                    for sub in range(2):
                        off = sub * 64
                        nc.tensor.matmul(
                            s[:, sub * 512 : (sub + 1) * 512],
                            KT[t][off : off + 64, kc * 128 : (kc + 1) * 128],
                            QT[t][off : off + 64, :],
                            start=True,
                            stop=True,
                        )
                    p = ppool.tile([128, 1024], BF16, tag="p", name="p")
                    nc.scalar.activation(
                        p[:], s[:], AF.Exp, bias=mb_sb[:, kc : kc + 1], scale=SCALE
                    )
                    while fi < len(fills) and fills[fi][0] <= kc:
                        fills[fi][1]()
                        fi += 1
                    # start marks the whole 2KB psum zero-region pending-zero,
                    # so only the first matmul into each bank starts and only
                    # the last stops; intermediate kc==0 qt>0 writes overwrite
                    # their (pending-zero) bytes automatically.
                    for sub in range(2):
                        for qt in range(NLQ):
                            nc.tensor.matmul(
                                o_ps[sub][:, qt, :],
                                p[:, sub * 512 + qt * 128 : sub * 512 + (qt + 1) * 128],
                                V_il[kc][:, 2 * t + sub, :],
                                start=(kc == 0 and qt == 0),
                                stop=(kc == nkvc - 1 and qt == NLQ - 1),
                            )
                while fi < len(fills):
                    fills[fi][1]()
                    fi += 1
                emit_norm(t, o_ps)
                if t == NH - 1:
                    emit_otrans(t, 0)
                    emit_otrans(t, 1)
                    # phase C: last Wo slice + combine + store. Adds split
                    # across DVE and Pool; C psums alternate between the
                    # (now idle) S and fill psum pools; stores batched into
                    # two 4-block DMAs on separate queues.
                    fin_g = [
                        finpool.tile(
                            [128, 2, 512], BF16, tag=f"fin{g}", name="fin", bufs=1
                        )
                        for g in range(4)
                    ]
                    out_q = [nc.sync, nc.scalar]
                    for mt in range(NH):
                        if mt % 2 == 0:
                            ps = spsum.tile([128, 1024], F32, tag="sps", name="ps")
                            ps = ps[:, 0:512]
                        else:
                            ps = fpsum.tile([128, 512], F32, tag="fp", name="ps")
                        nc.tensor.matmul(
                            ps[:],
                            wo_bf[7][:, mt * 128 : (mt + 1) * 128],
                            OT[7][:],
                            start=True,
                            stop=not use_fp8,
                        )
                        # fold the partial-sum combine into the psum group
                        # via an fp32r identity matmul (1 cycle/row at this
                        # width) so the evacuation is a copy, not an add,
                        # and can split across ScalarE and DVE
                        if use_fp8:
                            nc.tensor.matmul(
                                ps[:],
                                idf128[:],
                                outpart[mt][:],
                                start=False,
                                stop=True,
                            )
                        with nc.allow_low_precision(reason="bf16 output store"):
                            if not use_fp8:
                                nc.vector.tensor_tensor(
                                    fin_g[mt // 2][:, mt % 2, :],
                                    ps[:],
                                    outpart[mt][:],
                                    mybir.AluOpType.add,
                                )
                            elif mt % 2 == 0:
                                nc.scalar.copy(
                                    fin_g[mt // 2][:, mt % 2, :], ps[:]
                                )
                            else:
                                nc.vector.tensor_copy(
                                    fin_g[mt // 2][:, mt % 2, :], ps[:]
                                )
                        if mt % 2 == 1:
                            out_q[(mt // 2) % 2].dma_start(
                                row_block(out_d, mt - 1, mt + 1),
                                fin_g[mt // 2][:],
                            )

    nc.compile()
    return nc


_NC_CACHE = {}


def get_nc(nkvc=DEFAULT_NKVC):
    if nkvc not in _NC_CACHE:
        _NC_CACHE[nkvc] = build_nc(nkvc)
    return _NC_CACHE[nkvc]


def choose_nkvc(kv_mask):
    cnt = int(np.asarray(kv_mask).sum(axis=1).max())
    return max(1, -(-cnt // 128))


def make_in_maps(query, key_value, kv_mask, Wq, bq, Wk, bk, Wv, bv, Wo, bo,
                 nkvc=None):
    f = lambda x: np.ascontiguousarray(np.asarray(x), dtype=np.float32)
    bf = lambda x: np.ascontiguousarray(
        np.asarray(x, dtype=np.float32).astype(ml_dtypes.bfloat16)
    )
    kv_mask = np.asarray(kv_mask)
    if nkvc is None:
        nkvc = choose_nkvc(kv_mask)
    lkv = nkvc * 128
    query = bf(query)
    key_value = np.asarray(key_value, dtype=np.float32)
    Wo32 = f(Wo)
    bo_eff = (f(bv) @ Wo32 + f(bo)).astype(np.float32)
    def blk(w):  # [768, 1024] -> [8 blocks][128, (6*128)]
        return np.ascontiguousarray(
            bf(w).reshape(NQT, 128, NH, 128)
            .transpose(2, 1, 0, 3)
            .reshape(NH, 128, QD)
        )

    def hilo(w):
        # fp8e4 hi/lo split, stacked on a new leading axis. The weights are
        # pre-scaled by 64 so the lo residual stays clear of e4m3's
        # subnormal floor (W std is ~dim**-0.5); the kernel divides the
        # PSUM result by 64 during evacuation.
        w = f(w) * 64.0
        hi = w.astype(ml_dtypes.float8_e4m3)
        lo = (w - hi.astype(np.float32)).astype(ml_dtypes.float8_e4m3)
        return np.stack([hi, lo])

    common = {
        "ident": np.ascontiguousarray(
            np.eye(128, dtype=np.float32).astype(ml_dtypes.bfloat16)
        ),
        "Wo_bf": bf(Wo),
    }
    common["Wq_blk"] = blk(Wq)
    if nkvc <= 12:
        wk8 = hilo(Wk).reshape(2, NQT // 2, 2, 128, NH, 128)
        wv8 = hilo(Wv).reshape(2, NQT // 2, 2, 128, 2, 512)
        common["Wk_blk8"] = np.ascontiguousarray(wk8.transpose(4, 3, 0, 1, 2, 5))
        common["Wv_half8"] = np.ascontiguousarray(wv8.transpose(4, 3, 0, 1, 2, 5))
    else:
        common["Wk_blk"] = blk(Wk)
        wv_bf16 = bf(Wv).reshape(NQT, 128, 2, 512)
        common["Wv_half"] = np.ascontiguousarray(
            wv_bf16.transpose(2, 1, 0, 3).reshape(2, 128, NQT * 512)
        )
    bq8 = f(bq).reshape(NH, 128)
    bk8 = f(bk).reshape(NH, 128)
    boe8 = bo_eff.reshape(NH, 128)
    in_maps = []
    for b in range(B):
        idx = np.flatnonzero(kv_mask[b])
        kv_c = np.zeros((lkv, QD), dtype=ml_dtypes.bfloat16)
        kv_c[: idx.size] = key_value[b, idx].astype(ml_dtypes.bfloat16)
        mask_bias = np.full((lkv,), MASK_NEG, dtype=np.float32)
        mask_bias[: idx.size] = 0.0
        m = dict(common)
        m["q_bf"] = query[b]
        m["kv_bf"] = kv_c
        m["biasvec"] = np.ascontiguousarray(
            np.concatenate(
                [bq8, mask_bias.reshape(nkvc, 128), bk8, boe8], axis=0
            )
        )
        in_maps.append(m)
    return in_maps


def kernel(**inputs) -> np.ndarray:
    nkvc = choose_nkvc(inputs["kv_mask"])
    nc = get_nc(nkvc)
    in_maps = make_in_maps(**inputs, nkvc=nkvc)
    res = run_bass_kernel_spmd(nc, in_maps, core_ids=list(range(B)))
    out = np.stack([res.results[i]["out"].T for i in range(B)])
    return np.ascontiguousarray(out.astype(np.float32))
